# revision 22
# baseline (speedup 1.0000x reference)
"""Cabasc-style attention kernel for Trainium2 (Bass/Tile), 8-core data-parallel.

Contract: kernel(**inputs) takes the FULL unsharded inputs (as produced by the
problem's setup_inputs) and returns the FULL [128, 3] float32 output.

Sharding: data-parallel over the batch dim (16 rows per NeuronCore); the
embedding table and dense weights are replicated into each core's input map.

Math (per batch row b):
    len_b   = #nonzero(text_idx[b]);  alen_b = #nonzero(asp_idx[b])
    v_a     = sum_j E[asp_idx[b,j]] / alen_b
    m'_s    = E[text_idx[b,s]] * relu(1 - s/len_b)          (= masked+decayed memory)
    v_s     = sum_s E[text_idx[b,s]] * [s<len_b] / len_b
    c       = v_a @ W1a + v_s @ W1s + b1
    z.T     = W1m.T @ m'.T ;  score_s = w2 . tanh(z_s + c)
    alpha   = softmax(score);  v_ts = sum_s alpha_s m'_s
    out     = tanh((v_ts + v_s) @ Wm + bm) @ Wd + bd
"""

from contextlib import ExitStack

import numpy as np

import concourse.bass as bass
import concourse.tile as tile
from concourse import bacc, mybir
from concourse.bass_utils import run_bass_kernel_spmd

AF = mybir.ActivationFunctionType
ALU = mybir.AluOpType
AX = mybir.AxisListType
F32 = mybir.dt.float32
I32 = mybir.dt.int32

V, E, S, A = 50000, 300, 512, 8
B, NCORES = 128, 8
NB = B // NCORES              # 16 batch rows per core
H = 300                       # attention hidden dim (W1 output)
PO = 3                        # output dim
ECH = [(0, 128), (128, 128), (256, 44)]   # chunking of the E (=H) axis
SCH = S // 128                # 4 token chunks per batch row
NT = NB * SCH                 # 64 token tiles of 128 per core

_CACHE: dict = {}
DEBUG_TAPS = False


def _build_program():
    nc = bacc.Bacc(
        "TRN2", target_bir_lowering=False, debug=False, enable_asserts=False
    )

    def din(name, shape, dt=F32):
        return nc.dram_tensor(name, shape, dt, kind="ExternalInput").ap()

    embed = din("embed", [V, E])
    idx_cols = din("idx_cols", [128, NT], I32)
    idx_rows = din("idx_rows", [NB, S], I32)
    asp_col = din("asp_col", [128, 1], I32)
    asp_rows = din("asp_rows", [NB, A], I32)
    w1m_d = din("w1m", [E, H])
    w1a_d = din("w1a", [E, H])
    w1s_d = din("w1s", [E, H])
    wm_d = din("wm", [E, E])
    wd_d = din("wd", [E, PO])
    w2b_d = din("w2blk", [128, 3 * NB * NB])
    b1_d = din("b1r", [1, H])
    bm_d = din("bmr", [1, E])
    bd_d = din("bdr", [1, PO])
    bmask_d = din("blockmask", [128, NB])
    ident_d = din("ident", [128, 128])
    ones_d = din("ones16", [1, NB])
    out_d = nc.dram_tensor("out", [NB, PO], F32, kind="ExternalOutput").ap()
    dbg = {}
    if DEBUG_TAPS:
        for name, shape in [
            ("dbg_rawg0", [128, E]),
            ("dbg_decay", [NB, S]),
            ("dbg_coefS", [NB, S]),
            ("dbg_vs", [NB, E]),
            ("dbg_va", [NB, E]),
            ("dbg_c", [NB, H]),
            ("dbg_alpha", [NB, S]),
            ("dbg_vts", [NB, E]),
        ]:
            dbg[name] = nc.dram_tensor(name, shape, F32, kind="ExternalOutput").ap()

    with tile.TileContext(nc) as tc, ExitStack() as ctx:
        cp = ctx.enter_context(tc.tile_pool(name="cp", bufs=1))
        rawp = ctx.enter_context(tc.tile_pool(name="rawp", bufs=1))
        mtp = ctx.enter_context(tc.tile_pool(name="mtp", bufs=6))
        thp = ctx.enter_context(tc.tile_pool(name="thp", bufs=3))
        scr = ctx.enter_context(tc.tile_pool(name="scr", bufs=2))
        p_sc = ctx.enter_context(tc.tile_pool(name="p_sc", bufs=1, space="PSUM"))
        p_acc = ctx.enter_context(tc.tile_pool(name="p_acc", bufs=2, space="PSUM"))
        p_zt = ctx.enter_context(tc.tile_pool(name="p_zt", bufs=3, space="PSUM"))
        p_tp = ctx.enter_context(tc.tile_pool(name="p_tp", bufs=2, space="PSUM"))

        # ---- constants / weights into SBUF ----
        def load(name, src, shape, dt=F32):
            t = cp.tile(shape, dt, tag=name)
            nc.sync.dma_start(t[:], src)
            return t

        ident = load("ident", ident_d[:, :], [128, 128])
        bmask = load("bmask", bmask_d[:, :], [128, NB])
        w2b = load("w2b", w2b_d[:, :], [128, 3 * NB * NB])
        ones16 = load("ones16", ones_d[:, :], [1, NB])
        b1r = load("b1r", b1_d[:, :], [1, H])
        bmr = load("bmr", bm_d[:, :], [1, E])
        bdr = load("bdr", bd_d[:, :], [1, PO])
        idxc = load("idxc", idx_cols[:, :], [128, NT], I32)
        idxr = load("idxr", idx_rows[:, :], [NB, S], I32)
        aspc = load("aspc", asp_col[:, :], [128, 1], I32)
        aspr = load("aspr", asp_rows[:, :], [NB, A], I32)

        w1m_t, w1a_t, w1s_t, wm_t, wd_t = [], [], [], [], []
        for k, (e0, ec) in enumerate(ECH):
            w1m_t.append(load(f"w1m{k}", w1m_d[e0 : e0 + ec, :], [ec, H]))
            w1a_t.append(load(f"w1a{k}", w1a_d[e0 : e0 + ec, :], [ec, H]))
            w1s_t.append(load(f"w1s{k}", w1s_d[e0 : e0 + ec, :], [ec, H]))
            wm_t.append(load(f"wm{k}", wm_d[e0 : e0 + ec, :], [ec, E]))
            wd_t.append(load(f"wd{k}", wd_d[e0 : e0 + ec, :], [ec, PO]))

        id16 = ident[:NB, :NB]

        # ---- gathers (issue early; Tile overlaps everything downstream) ----
        aspg = cp.tile([128, E], F32, tag="aspg")
        nc.gpsimd.indirect_dma_start(
            out=aspg[:],
            out_offset=None,
            in_=embed[:, :],
            in_offset=bass.IndirectOffsetOnAxis(ap=aspc[:, :1], axis=0),
        )
        raw = []
        for t in range(NT):
            rt = rawp.tile([128, E], F32, tag=f"g{t}")
            nc.gpsimd.indirect_dma_start(
                out=rt[:],
                out_offset=None,
                in_=embed[:, :],
                in_offset=bass.IndirectOffsetOnAxis(ap=idxc[:, t : t + 1], axis=0),
            )
            raw.append(rt)

        def tok(t):  # token tile t -> [128, E]
            return raw[t][:]

        if DEBUG_TAPS:
            nc.sync.dma_start(dbg["dbg_rawg0"][:, :], raw[0][:])

        # ---- lengths, decay, coefficients ----
        idxf = cp.tile([NB, S], F32, tag="idxf")
        nc.vector.tensor_copy(idxf[:], idxr[:])
        zc = cp.tile([NB, S], F32, tag="zc")
        nc.vector.tensor_scalar(zc[:], idxf[:], 0.0, None, op0=ALU.is_equal)
        nz = cp.tile([NB, 1], F32, tag="nz")
        nc.vector.reduce_sum(nz[:], zc[:], axis=AX.X)
        len_f = cp.tile([NB, 1], F32, tag="len_f")
        nc.scalar.activation(len_f[:], nz[:], AF.Copy, bias=float(S), scale=-1.0)
        inv_len = cp.tile([NB, 1], F32, tag="inv_len")
        nc.vector.reciprocal(inv_len[:], len_f[:])
        neg_inv = cp.tile([NB, 1], F32, tag="neg_inv")
        nc.vector.tensor_scalar_mul(neg_inv[:], inv_len[:], -1.0)

        aspf = cp.tile([NB, A], F32, tag="aspf")
        nc.vector.tensor_copy(aspf[:], aspr[:])
        azc = cp.tile([NB, A], F32, tag="azc")
        nc.vector.tensor_scalar(azc[:], aspf[:], 0.0, None, op0=ALU.is_equal)
        anz = cp.tile([NB, 1], F32, tag="anz")
        nc.vector.reduce_sum(anz[:], azc[:], axis=AX.X)
        alen = cp.tile([NB, 1], F32, tag="alen")
        nc.scalar.activation(alen[:], anz[:], AF.Copy, bias=float(A), scale=-1.0)
        rasp = cp.tile([NB, 1], F32, tag="rasp")
        nc.vector.reciprocal(rasp[:], alen[:])

        posi = cp.tile([NB, S], I32, tag="posi")
        nc.gpsimd.iota(posi[:], pattern=[[1, S]], base=0, channel_multiplier=0)
        posf = cp.tile([NB, S], F32, tag="posf")
        nc.vector.tensor_copy(posf[:], posi[:])

        decay = cp.tile([NB, S], F32, tag="decay")
        nc.vector.tensor_scalar(
            decay[:], posf[:], neg_inv[:, :1], 1.0, op0=ALU.mult, op1=ALU.add
        )
        nc.vector.tensor_scalar_max(decay[:], decay[:], 0.0)
        coefS = cp.tile([NB, S], F32, tag="coefS")
        nc.vector.tensor_scalar(coefS[:], posf[:], len_f[:, :1], None, op0=ALU.is_lt)
        nc.vector.tensor_scalar(
            coefS[:], coefS[:], inv_len[:, :1], None, op0=ALU.mult
        )

        # transpose [NB,128] row chunks -> [128,NB] column tiles (PE matmul w/ id16)
        def to_cols(src_row, tagbase):
            cols = []
            for j in range(SCH):
                tp = p_tp.tile([128, 512], F32, tag="tp")
                nc.tensor.matmul(
                    tp[:, :NB],
                    lhsT=src_row[:, j * 128 : (j + 1) * 128],
                    rhs=id16,
                    start=True,
                    stop=True,
                )
                c = cp.tile([128, NB], F32, tag=f"{tagbase}{j}")
                nc.vector.tensor_copy(c[:], tp[:, :NB])
                cols.append(c)
            return cols

        dcol = to_cols(decay, "dcol")
        scol = to_cols(coefS, "scol")
        if DEBUG_TAPS:
            nc.sync.dma_start(dbg["dbg_decay"][:, :], decay[:])
            nc.sync.dma_start(dbg["dbg_coefS"][:, :], coefS[:])

        # ---- v_a ----
        vap = p_acc.tile([NB, E], F32, tag="acc")
        nc.tensor.matmul(vap[:], lhsT=bmask[:], rhs=aspg[:], start=True, stop=True)
        va = cp.tile([NB, E], F32, tag="va")
        nc.vector.tensor_scalar(va[:], vap[:], rasp[:, :1], None, op0=ALU.mult)

        # ---- v_s (PE matvecs on raw embeddings, coef = mask/len) ----
        vs_flat = cp.tile([1, NB * E], F32, tag="vs_flat")
        for b in range(NB):
            vsp = p_acc.tile([1, E], F32, tag="acc")
            for j in range(SCH):
                nc.tensor.matmul(
                    vsp[:, :],
                    lhsT=scol[j][:, b : b + 1],
                    rhs=tok(b * SCH + j),
                    start=(j == 0),
                    stop=(j == SCH - 1),
                )
            nc.scalar.copy(vs_flat[:, b * E : (b + 1) * E], vsp[:, :])
        vs = cp.tile([NB, E], F32, tag="vs")
        nc.sync.dma_start(vs[:, :], vs_flat[:, :])
        if DEBUG_TAPS:
            nc.sync.dma_start(dbg["dbg_vs"][:, :], vs[:])
            nc.sync.dma_start(dbg["dbg_va"][:, :], va[:])

        # ---- scale raw tiles in place by decay ----
        for t in range(NT):
            b, j = t // SCH, t % SCH
            nc.vector.tensor_scalar(
                tok(t), tok(t), dcol[j][:, b : b + 1], None, op0=ALU.mult
            )

        # ---- c = v_a @ W1a + v_s @ W1s + b1  -> cT chunks [H_c, NB] ----
        def row_to_chunkT(src_row, tagbase, width=E):
            outs = []
            for k, (e0, ec) in enumerate(ECH):
                tp = p_tp.tile([128, 512], F32, tag="tp")
                nc.tensor.matmul(
                    tp[:ec, :NB],
                    lhsT=src_row[:, e0 : e0 + ec],
                    rhs=id16,
                    start=True,
                    stop=True,
                )
                c = cp.tile([ec, NB], F32, tag=f"{tagbase}{k}")
                nc.vector.tensor_copy(c[:], tp[:ec, :NB])
                outs.append(c)
            return outs

        vaT = row_to_chunkT(va, "vaT")
        vsT = row_to_chunkT(vs, "vsT")
        cps = p_acc.tile([NB, H], F32, tag="acc")
        nc.tensor.matmul(cps[:], lhsT=ones16[:], rhs=b1r[:], start=True, stop=False)
        for k in range(3):
            nc.tensor.matmul(
                cps[:], lhsT=vaT[k][:], rhs=w1a_t[k][:], start=False, stop=False
            )
        for k in range(3):
            nc.tensor.matmul(
                cps[:], lhsT=vsT[k][:], rhs=w1s_t[k][:], start=False, stop=(k == 2)
            )
        c_sb = cp.tile([NB, H], F32, tag="c_sb")
        nc.vector.tensor_copy(c_sb[:], cps[:])
        cT = row_to_chunkT(c_sb, "cT")
        if DEBUG_TAPS:
            nc.sync.dma_start(dbg["dbg_c"][:, :], c_sb[:])

        # ---- big loop over batch rows: z.T -> tanh -> scores ----
        scores = p_sc.tile([NB, S], F32, tag="sc")
        for b in range(NB):
            mt = []
            for k, (e0, ec) in enumerate(ECH):
                tp = p_tp.tile([128, 512], F32, tag="tp")
                for j in range(SCH):
                    nc.tensor.matmul(
                        tp[:ec, j * 128 : (j + 1) * 128],
                        lhsT=tok(b * SCH + j)[:, e0 : e0 + ec],
                        rhs=ident[:, :],
                        start=True,
                        stop=True,
                    )
                m = mtp.tile([128, S], F32, tag="mt")
                nc.vector.tensor_copy(m[:ec, :], tp[:ec, :])
                mt.append(m)
            for i, (h0, hc) in enumerate(ECH):
                zt = p_zt.tile([128, S], F32, tag="zt")
                for k, (e0, ec) in enumerate(ECH):
                    nc.tensor.matmul(
                        zt[:hc, :],
                        lhsT=w1m_t[k][:, h0 : h0 + hc],
                        rhs=mt[k][:ec, :],
                        start=(k == 0),
                        stop=(k == 2),
                    )
                th = thp.tile([128, S], F32, tag="th")
                nc.scalar.activation(
                    th[:hc, :], zt[:hc, :], AF.Tanh, bias=cT[i][:, b : b + 1]
                )
                blk = (i * NB + b) * NB
                nc.tensor.matmul(
                    scores[:, :],
                    lhsT=w2b[:hc, blk : blk + NB],
                    rhs=th[:hc, :],
                    start=(b == 0 and i == 0),
                    stop=(b == NB - 1 and i == 2),
                )

        # ---- softmax over S (rows = batch) ----
        mx = scr.tile([NB, 1], F32, tag="mx")
        nc.vector.reduce_max(mx[:], scores[:], axis=AX.X)
        negmx = scr.tile([NB, 1], F32, tag="negmx")
        nc.vector.tensor_scalar_mul(negmx[:], mx[:], -1.0)
        alpha = cp.tile([NB, S], F32, tag="alpha")
        sumexp = scr.tile([NB, 1], F32, tag="sumexp")
        nc.scalar.activation(
            alpha[:], scores[:], AF.Exp, bias=negmx[:, :1], accum_out=sumexp[:, :1]
        )
        rsum = scr.tile([NB, 1], F32, tag="rsum")
        nc.vector.reciprocal(rsum[:], sumexp[:])
        nc.vector.tensor_scalar(alpha[:], alpha[:], rsum[:, :1], None, op0=ALU.mult)

        acol = to_cols(alpha, "acol")
        if DEBUG_TAPS:
            nc.sync.dma_start(dbg["dbg_alpha"][:, :], alpha[:])

        # ---- v_ts (PE matvecs on scaled tiles, coef = alpha) ----
        vts_flat = cp.tile([1, NB * E], F32, tag="vts_flat")
        for b in range(NB):
            vtp = p_acc.tile([1, E], F32, tag="acc")
            for j in range(SCH):
                nc.tensor.matmul(
                    vtp[:, :],
                    lhsT=acol[j][:, b : b + 1],
                    rhs=tok(b * SCH + j),
                    start=(j == 0),
                    stop=(j == SCH - 1),
                )
            nc.vector.tensor_copy(vts_flat[:, b * E : (b + 1) * E], vtp[:, :])
        vts = cp.tile([NB, E], F32, tag="vts")
        nc.sync.dma_start(vts[:, :], vts_flat[:, :])
        if DEBUG_TAPS:
            nc.sync.dma_start(dbg["dbg_vts"][:, :], vts[:])

        # ---- tail MLP ----
        vns = cp.tile([NB, E], F32, tag="vns")
        nc.vector.tensor_add(vns[:], vts[:], vs[:])
        vnsT = row_to_chunkT(vns, "vnsT")
        vmp = p_acc.tile([NB, E], F32, tag="acc")
        nc.tensor.matmul(vmp[:], lhsT=ones16[:], rhs=bmr[:], start=True, stop=False)
        for k in range(3):
            nc.tensor.matmul(
                vmp[:], lhsT=vnsT[k][:], rhs=wm_t[k][:], start=False, stop=(k == 2)
            )
        vms = cp.tile([NB, E], F32, tag="vms")
        nc.scalar.activation(vms[:], vmp[:], AF.Tanh)
        vmsT = row_to_chunkT(vms, "vmsT")
        op = p_acc.tile([NB, PO], F32, tag="acc")
        nc.tensor.matmul(op[:], lhsT=ones16[:], rhs=bdr[:], start=True, stop=False)
        for k in range(3):
            nc.tensor.matmul(
                op[:], lhsT=vmsT[k][:], rhs=wd_t[k][:], start=False, stop=(k == 2)
            )
        out_sb = cp.tile([NB, PO], F32, tag="out_sb")
        nc.vector.tensor_copy(out_sb[:], op[:])
        nc.sync.dma_start(out_d[:, :], out_sb[:])

    nc.compile()
    return nc


def _prep_in_maps(inputs):
    ti = np.ascontiguousarray(np.asarray(inputs["text_raw_indices"]).astype(np.int32))
    ai = np.ascontiguousarray(np.asarray(inputs["aspect_indices"]).astype(np.int32))
    emb = np.ascontiguousarray(np.asarray(inputs["embed"], dtype=np.float32))
    W1 = np.asarray(inputs["W1"], dtype=np.float32)
    w2 = np.asarray(inputs["w2"], dtype=np.float32)
    Wm = np.ascontiguousarray(np.asarray(inputs["Wm"], dtype=np.float32))
    Wd = np.ascontiguousarray(np.asarray(inputs["Wd"], dtype=np.float32))
    b1 = np.asarray(inputs["b1"], dtype=np.float32)
    bm = np.asarray(inputs["bm"], dtype=np.float32)
    bd = np.asarray(inputs["bd"], dtype=np.float32)

    w2blk = np.zeros((128, 3 * NB * NB), np.float32)
    for i in range(3):
        e0, ec = ECH[i]
        for b in range(NB):
            w2blk[:ec, (i * NB + b) * NB + b] = w2[e0 : e0 + ec]
    bmask = np.zeros((128, NB), np.float32)
    bmask[np.arange(128), np.arange(128) // A] = 1.0
    shared = {
        "embed": emb,
        "w1m": np.ascontiguousarray(W1[:E]),
        "w1a": np.ascontiguousarray(W1[E : 2 * E]),
        "w1s": np.ascontiguousarray(W1[2 * E :]),
        "wm": Wm,
        "wd": Wd,
        "w2blk": w2blk,
        "b1r": b1.reshape(1, H),
        "bmr": bm.reshape(1, E),
        "bdr": bd.reshape(1, PO),
        "blockmask": bmask,
        "ident": np.eye(128, dtype=np.float32),
        "ones16": np.ones((1, NB), np.float32),
    }
    in_maps = []
    for c in range(NCORES):
        rows = ti[c * NB : (c + 1) * NB]                        # [16, 512]
        icols = np.ascontiguousarray(
            rows.reshape(NB, SCH, 128).transpose(2, 0, 1).reshape(128, NT)
        )
        arows = ai[c * NB : (c + 1) * NB]                       # [16, 8]
        in_maps.append(
            dict(
                shared,
                idx_cols=icols,
                idx_rows=np.ascontiguousarray(rows),
                asp_col=np.ascontiguousarray(arows.reshape(128, 1)),
                asp_rows=np.ascontiguousarray(arows),
            )
        )
    return in_maps


def get_program():
    if "nc" not in _CACHE:
        _CACHE["nc"] = _build_program()
    return _CACHE["nc"]


def run(inputs, **spmd_kwargs):
    nc = get_program()
    in_maps = _prep_in_maps(inputs)
    res = run_bass_kernel_spmd(nc, in_maps, core_ids=list(range(NCORES)), **spmd_kwargs)
    out = np.concatenate([res.results[c]["out"] for c in range(NCORES)], axis=0)
    return out.astype(np.float32), res


def kernel(**inputs):
    out, _ = run(inputs)
    return out
